# revision 26
# baseline (speedup 1.0000x reference)
"""3-layer GAT + MLP head on trn2, node-sharded across 8 NeuronCores.

Strategy: dst nodes partitioned 8 ways (6250/core, padded to 6272). Layer 1's
transform is computed REDUNDANTLY on every core straight from the replicated
input x (writes the full payload tables locally - no collective, no startup
stall). Layers 2/3 replicate their payload tables with TWO AllGathers per
layer (source rows split 25/24 blocks per core into tables A and B), issued
mid-sweep so they always overlap edge compute.

The edge phase is q-major two-sweep: sweep 1 handles all table-A edge groups,
accumulating one-hot scatter matmuls into an SBUF f32 accumulator; sweep 2
handles table-B groups, finalizes each dst block (softmax denominators ride
the scatter matmul; normalization is a per-partition scaled relu on the
scalar engine), and fuses the NEXT layer's transform + staging + AllGather
triggers. Next layer's sweep 1 needs only table A (long landed) and its work
covers AllGather-B still in flight.

Edge groups span PAIRS of dst blocks (super-groups) to halve per-group
instruction overhead; gather calls span the pair, capped at 8 sub-blocks
(1024 descriptors, the SWDGE ring limit). Within each (core,q,block) group
edges are sorted by source row so the hardware dma_gather walks HBM in
ascending order. Per-edge dst logits come from a transposed one-hot matmul
(host-shipped ohT) whose outputs ride spare columns of the same PSUM banks
as the scatter accumulation.

Features use a head-minor (c,h) layout (f = c*4 + h, via host-side weight
permutations) so the per-edge em*h multiply has a contiguous innermost run
of 4 bf16 values and qualifies for the DVE 2x performance mode.
"""
import sys, os, types
sys.path.insert(0, '/opt/trn_rl_repo')
import numpy as np
import concourse.bass as bass
import concourse.bacc as bacc
import concourse.tile as tile
from concourse import mybir
from concourse import bass_utils
from concourse.bass_utils import run_bass_kernel_spmd
from concourse.masks import make_identity

N = 50000
F0 = 128
HID = 64
H = 4
DH = 256          # H*HID
OUTD = 40
NEG = 0.2
NC8 = 8
SH = 6250         # dst nodes per core
NB = 49           # 128-node blocks per core
SHP = NB * 128    # 6272
NBA = 25          # blocks in stage half A
NBB = 24          # blocks in stage half B
SPA = NBA * 128   # 3200 rows per core in half A
SPB = NBB * 128   # 3072
ROWSA = NC8 * SPA  # 25600 rows in table A (int16-indexable)
ROWSB = NC8 * SPB  # 24576
FP8 = False       # fp8 payload rows (512B) vs bf16 (768B)
PAYW = 512 if FP8 else 384   # payload row width in elements (%256B rule)
PAYU = 280 if FP8 else 264   # staged columns: h 256 | em slot | sl bits
SLO = 264 if FP8 else 256    # payload col offset of sl (f32 bits)
SLW = 16 if FP8 else 8       # sl width in payload elements
SCATW = 260       # scatter-matmul rhs window: h*em(256) | em(4)
RHSW = 264        # transform psum window: h(256) | sl(4) | dl(4)
MAXSB = 8         # sub-blocks per gather call (1024 descs = hard ring cap)
DLC = 320         # f32 col offset of dl results inside each scatter PSUM bank

LAST_EXEC_NS = None


def _install_ntff_hook():
    if "antenv.axon_hooks" in sys.modules:
        return
    try:
        import antenv
        from trn_agent_boot.trn_boot import _ntff_profile_via_ctypes
        hook = _ntff_profile_via_ctypes('/opt/axon/libaxon_pjrt.so')
    except Exception:
        hook = None
    m = types.ModuleType("antenv.axon_hooks")
    m.get_axon_ntff_profile_hook = lambda: hook
    m.set_axon_ntff_profile_hook = lambda h: None
    sys.modules["antenv.axon_hooks"] = m
    bass_utils.upload_artifacts = lambda d: f"local:{d}"


PAIRS = [(k, k + 1) for k in range(0, NB - 1, 2)] + [(NB - 1,)]


def _prep_edges(edge_index):
    src = np.asarray(edge_index[0], dtype=np.int64)
    dst = np.asarray(edge_index[1], dtype=np.int64)
    loop = np.arange(N, dtype=np.int64)
    src = np.concatenate([src, loop])
    dst = np.concatenate([dst, loop])

    core = dst // SH
    ldst = dst - core * SH
    blk = ldst // 128
    dloc = (ldst - blk * 128).astype(np.float32)
    scr = src // SH
    r = src - scr * SH
    q = (r >= SPA).astype(np.int64)
    rel = np.where(q == 0, scr * SPA + r, scr * SPB + (r - SPA)).astype(np.int16)

    # uniform (over cores) sub-block counts per (q, k)
    key = core * (2 * NB) + q * NB + blk
    counts = np.bincount(key, minlength=NC8 * 2 * NB).reshape(NC8, 2, NB)
    s_max = np.ceil(counts.max(axis=0) / 128).astype(np.int64)  # [2, NB]
    s_max = np.maximum(s_max, 1)
    # one extra all-pad sub-block on block 48 (q=0): carries fake self-edges
    # for the 22 pad dst slots so their softmax denominators stay finite (a
    # NaN there would poison the dl matmul contraction for the whole block)
    s_max[0][NB - 1] += 1

    # slot layout: pair-major, then q, then block-within-pair, so the two
    # blocks of a super-group have contiguous sub-block slots per q
    base_qk = np.zeros((2, NB), dtype=np.int64)
    acc = 0
    for pr in PAIRS:
        for qq in range(2):
            for k in pr:
                base_qk[qq][k] = acc
                acc += int(s_max[qq][k])
    SBT = int(acc)
    base_flat = np.zeros(2 * NB, dtype=np.int64)   # indexed by q*NB + k
    for qq in range(2):
        for k in range(NB):
            base_flat[qq * NB + k] = base_qk[qq][k]

    # sort within each (core,q,block) group by source row for HBM locality
    order = np.lexsort((rel, key))
    key_s = key[order]
    gcounts = np.bincount(key_s, minlength=NC8 * 2 * NB)
    gstart = np.zeros(NC8 * 2 * NB + 1, dtype=np.int64)
    gstart[1:] = np.cumsum(gcounts)
    pos = np.arange(len(key_s)) - gstart[key_s]
    qk = key_s % (2 * NB)
    slot = base_flat[qk] * 128 + pos
    core_s = key_s // (2 * NB)

    # empty slots gather row 0 (the ucode mishandles mid-call negative
    # indices, so -1 skipping is not usable); the dloc>=0 mask on alpha
    # still forces em=1 there so slot contents never compound
    pay_idx = np.zeros((NC8, SBT * 128), dtype=np.int16)
    dloc_a = np.full((NC8, SBT * 128), -1.0, dtype=np.float32)
    pay_idx[core_s, slot] = rel[order]
    dloc_a[core_s, slot] = dloc[order]
    # fake self-edges for block-48 pad dst slots (see s_max bump above)
    pad0 = (base_qk[0][NB - 1] + s_max[0][NB - 1] - 1) * 128
    npad = SHP - SH  # 22
    dloc_a[:, pad0:pad0 + npad] = np.arange(128 - npad, 128, dtype=np.float32)
    pay_idx[:, pad0:pad0 + npad] = 0

    # wrapped int16 idx layout: idx i of a call at [i%16, i//16], replicated
    # across the 8 gpsimd-core stripes of 16 partitions each
    idxP = np.ascontiguousarray(np.tile(
        pay_idx.reshape(NC8, SBT * 8, 16).transpose(0, 2, 1), (1, 8, 1)))
    dlocT = np.ascontiguousarray(
        dloc_a.reshape(NC8, SBT, 128).transpose(0, 2, 1))  # [NC8, 128, SBT]
    # host-built transposed one-hot (partition = dst slot, free = edge slot),
    # bf16 0/1: the lhsT operand of the per-edge dl matmul
    bfdt = mybir.dt.np(mybir.dt.bfloat16)
    ohT = (dloc_a.reshape(NC8, SBT, 128)[:, None, :, :]
           == np.arange(128, dtype=np.float32)[None, :, None, None])
    ohT = np.ascontiguousarray(ohT).astype(bfdt).reshape(NC8, 128, SBT * 128)
    return s_max, base_qk, SBT, idxP, dlocT, ohT


def _pack_attn(a_s, a_d):
    p_s = np.zeros((DH, H), dtype=np.float32)
    p_d = np.zeros((DH, H), dtype=np.float32)
    for h in range(H):
        p_s[h * HID:(h + 1) * HID, h] = a_s[h]
        p_d[h * HID:(h + 1) * HID, h] = a_d[h]
    return p_s, p_d


def _build(s_max, base_qk, SBT):
    f32 = mybir.dt.float32
    bf16 = mybir.dt.bfloat16
    pdt = mybir.dt.float8e4 if FP8 else mybir.dt.bfloat16
    i16 = mybir.dt.int16
    AF = mybir.ActivationFunctionType
    nc = bacc.Bacc("TRN2", target_bir_lowering=False, debug=False,
                   num_swdge_queues=4)

    # max sub-blocks of a (q, pair) super-group
    S2MX = max(sum(int(s_max[qq][k]) for k in pr)
               for pr in PAIRS for qq in range(2))

    xT = nc.dram_tensor("xT", [F0, SHP], bf16, kind="ExternalInput")
    xT8 = nc.dram_tensor("xT8", [F0, NC8 * SHP], bf16, kind="ExternalInput")
    W1e = nc.dram_tensor("W1e", [F0, RHSW], bf16, kind="ExternalInput")
    W2e = nc.dram_tensor("W2e", [DH, RHSW], bf16, kind="ExternalInput")
    W3e = nc.dram_tensor("W3e", [DH, RHSW], bf16, kind="ExternalInput")
    Wm1 = nc.dram_tensor("Wm1", [DH, DH], bf16, kind="ExternalInput")
    Wm2 = nc.dram_tensor("Wm2", [DH, OUTD], bf16, kind="ExternalInput")
    idxP_d = nc.dram_tensor("idxP", [128, SBT * 8], i16, kind="ExternalInput")
    dlocT = nc.dram_tensor("dlocT", [128, SBT], f32, kind="ExternalInput")
    ohT_d = nc.dram_tensor("ohT", [128, SBT * 128], bf16,
                           kind="ExternalInput")
    iotaF = nc.dram_tensor("iotaF", [128, 128], f32, kind="ExternalInput")
    out = nc.dram_tensor("out", [SHP, OUTD], f32, kind="ExternalOutput")

    stageA = [nc.dram_tensor(f"stageA{p}", [SPA, PAYW], pdt) for p in range(2)]
    stageB = [nc.dram_tensor(f"stageB{p}", [SPB, PAYW], pdt) for p in range(2)]
    tabA = [nc.dram_tensor(f"tabA{p}", [ROWSA, PAYW], pdt,
                           addr_space="Shared") for p in range(2)]
    tabB = [nc.dram_tensor(f"tabB{p}", [ROWSB, PAYW], pdt,
                           addr_space="Shared") for p in range(2)]

    with tile.TileContext(nc) as tc:
        with tc.tile_pool(name="const", bufs=1) as cp, \
             tc.tile_pool(name="work", bufs=2) as wp, \
             tc.tile_pool(name="zt", bufs=1) as zp, \
             tc.tile_pool(name="psA", bufs=2, space="PSUM") as psA, \
             tc.tile_pool(name="psB", bufs=2, space="PSUM") as psB, \
             tc.tile_pool(name="psC", bufs=2, space="PSUM") as psC:

            from concourse import library_config
            ident = cp.tile([128, 128], bf16)
            make_identity(nc, ident[:])
            nc.gpsimd.load_library(library_config.mlp)
            iota_sb = cp.tile([128, 128], f32)
            nc.sync.dma_start(out=iota_sb[:], in_=iotaF[:])
            cNEG = cp.tile([128, 4], f32)
            nc.gpsimd.memset(cNEG[:], NEG)
            dloc_sb = cp.tile([128, SBT], f32)
            nc.sync.dma_start(out=dloc_sb[:], in_=dlocT[:])

            w1_sb = cp.tile([128, RHSW], bf16)
            nc.sync.dma_start(out=w1_sb[:], in_=W1e[:])
            w2_sb = [cp.tile([128, RHSW], bf16, tag=f"w2_{c}", name=f"w2_{c}")
                     for c in range(2)]
            w3_sb = [cp.tile([128, RHSW], bf16, tag=f"w3_{c}", name=f"w3_{c}")
                     for c in range(2)]
            wm1_sb = [cp.tile([128, DH], bf16, tag=f"wm1_{c}", name=f"wm1_{c}")
                      for c in range(2)]
            wm2_sb = [cp.tile([128, OUTD], bf16, tag=f"wm2_{c}", name=f"wm2_{c}")
                      for c in range(2)]
            for c in range(2):
                nc.sync.dma_start(out=w2_sb[c][:], in_=W2e[c*128:(c+1)*128, :])
                nc.sync.dma_start(out=w3_sb[c][:], in_=W3e[c*128:(c+1)*128, :])
                nc.sync.dma_start(out=wm1_sb[c][:], in_=Wm1[c*128:(c+1)*128, :])
                nc.sync.dma_start(out=wm2_sb[c][:], in_=Wm2[c*128:(c+1)*128, :])

            zt_x = zp.tile([128, SHP], bf16, tag="ztx", name="ztx")
            nc.sync.dma_start(out=zt_x[:], in_=xT[:])
            dl_all = [zp.tile([128, NB, 4], bf16, tag=f"dl{p}", name=f"dl{p}")
                      for p in range(2)]
            acc = zp.tile([128, NB, SCATW], f32, tag="acc", name="acc")

            qrr = [0]

            def stage_write(ps2, k, p, dl_tile):
                """Copy a transform PSUM block into payload staging + dl."""
                hb2 = wp.tile([128, PAYU], pdt, tag="hb2", bufs=3)
                nc.scalar.activation(out=hb2[:, 0:256], in_=ps2[:, 0:256],
                                     func=AF.Copy)
                nc.vector.tensor_copy(
                    out=hb2[:, SLO:SLO+SLW].bitcast(f32),
                    in_=ps2[:, 256:260])
                nc.vector.tensor_copy(out=dl_tile[:, k, :],
                                      in_=ps2[:, 260:264])
                if k < NBA:
                    nc.sync.dma_start(
                        out=stageA[p][k*128:(k+1)*128, 0:PAYU], in_=hb2[:])
                else:
                    kk = k - NBA
                    nc.sync.dma_start(
                        out=stageB[p][kk*128:(kk+1)*128, 0:PAYU], in_=hb2[:])

            def ag_a(p):
                nc.gpsimd.collective_compute(
                    "AllGather", mybir.AluOpType.bypass,
                    replica_groups=[list(range(NC8))],
                    ins=[stageA[p][:]], outs=[tabA[p][:]],
                )

            def ag_b(p):
                nc.gpsimd.collective_compute(
                    "AllGather", mybir.AluOpType.bypass,
                    replica_groups=[list(range(NC8))],
                    ins=[stageB[p][:]], outs=[tabB[p][:]],
                )

            def edge_group(pr, qq, tab, dl_tile, first):
                """Process the (q, block-pair) super-group: gather payload
                rows, per-edge softmax weights, scatter into acc."""
                ss = [int(s_max[qq][k]) for k in pr]
                s2 = sum(ss)
                b0 = int(base_qk[qq][pr[0]])
                ohTt = wp.tile([128, S2MX * 128], bf16, tag="ohT", bufs=2)
                nc.scalar.dma_start(
                    out=ohTt[:, 0:s2*128], in_=ohT_d[:, b0*128:(b0+s2)*128])
                ixp = wp.tile([128, S2MX * 8], i16, tag="ixp", bufs=4)
                nc.sync.dma_start(out=ixp[:, 0:s2*8],
                                  in_=idxP_d[:, b0*8:(b0+s2)*8])
                pay = wp.tile([128, S2MX, PAYW], pdt, tag="pay", bufs=4)
                ncalls = -(-s2 // MAXSB)
                s0 = 0
                for c in range(ncalls):
                    nblk = s2 // ncalls + (1 if c < s2 % ncalls else 0)
                    nc.gpsimd.dma_gather(
                        pay[:, s0:s0+nblk, :], tab[:],
                        ixp[:, s0*8:(s0+nblk)*8], nblk * 128,
                        nblk * 128, PAYW, queue_num=qrr[0] % 4)
                    qrr[0] += 1
                    s0 += nblk
                # per-edge dl via transposed one-hot matmul (own psum bank;
                # its spare space also hosts the finalize transposes)
                ps = psA.tile([128, 2, 512], f32, tag="eacc")
                dlp = psC.tile([128, 512], f32, tag="po")
                off = 0
                for i, k in enumerate(pr):
                    for j in range(ss[i]):
                        nc.tensor.matmul(
                            out=dlp[:, (off+j)*4:(off+j+1)*4],
                            lhsT=ohTt[:, (off+j)*128:(off+j+1)*128],
                            rhs=dl_tile[:, k, :], start=True, stop=True)
                    off += ss[i]
                # em = exp(leakyrelu(sl + dl)); lrelu on DVE
                alw = wp.tile([128, S2MX, 4], f32, tag="alw", bufs=2)
                al2 = wp.tile([128, S2MX, 4], f32, tag="al2", bufs=2)
                off = 0
                for i, k in enumerate(pr):
                    nc.vector.tensor_tensor(
                        out=alw[:, off:off+ss[i], :],
                        in0=pay[:, off:off+ss[i], SLO:SLO+SLW].bitcast(f32),
                        in1=dlp[:, off*4:(off+ss[i])*4].rearrange(
                            "p (j c) -> p j c", j=ss[i]),
                        op=mybir.AluOpType.add)
                    off += ss[i]
                nc.vector.tensor_tensor(
                    out=al2[:, 0:s2, :], in0=alw[:, 0:s2, :],
                    in1=cNEG[:, 0:1, None].to_broadcast([128, s2, 4]),
                    op=mybir.AluOpType.mult)
                nc.vector.tensor_tensor(
                    out=alw[:, 0:s2, :], in0=alw[:, 0:s2, :],
                    in1=al2[:, 0:s2, :], op=mybir.AluOpType.max)
                nc.scalar.activation(
                    out=pay[:, 0:s2, 256:260], in_=alw[:, 0:s2, :],
                    func=AF.Exp)
                # head-minor (c,h) layout: em broadcast has a contiguous
                # 4-wide innermost run -> DVE 2x mode
                pay4 = pay[:, 0:s2, 0:DH].rearrange(
                    "p j (c h) -> p j c h", h=H)
                nc.vector.tensor_tensor(
                    out=pay4, in0=pay4,
                    in1=pay[:, 0:s2, None, 256:260].to_broadcast(
                        [128, s2, HID, H]),
                    op=mybir.AluOpType.mult)
                ohw = wp.tile([128, S2MX, 128], bf16, tag="ohw", bufs=2)
                nc.vector.tensor_tensor(
                    out=ohw[:, 0:s2, :],
                    in0=dloc_sb[:, b0:b0+s2, None].to_broadcast([128, s2, 128]),
                    in1=iota_sb[:, None, :].to_broadcast([128, s2, 128]),
                    op=mybir.AluOpType.is_equal)
                off = 0
                for i, k in enumerate(pr):
                    for j in range(ss[i]):
                        nc.tensor.matmul(
                            out=ps[:, i, 0:SCATW], lhsT=ohw[:, off+j, :],
                            rhs=pay[:, off+j, 0:SCATW],
                            start=(j == 0), stop=(j == ss[i] - 1))
                    off += ss[i]
                    if first:
                        nc.scalar.activation(out=acc[:, k, :],
                                             in_=ps[:, i, 0:SCATW],
                                             func=AF.Copy)
                    else:
                        nc.vector.tensor_tensor(
                            out=acc[:, k, :], in0=acc[:, k, :],
                            in1=ps[:, i, 0:SCATW], op=mybir.AluOpType.add)
                return dlp

            def finalize(k, dlp):
                """z = relu(acc_h * (1/denom_h)); returns zk transposed chunks."""
                rec = wp.tile([128, 4], f32, tag="rec", bufs=3)
                nc.vector.reciprocal(out=rec[:], in_=acc[:, k, 256:260])
                z = wp.tile([128, DH], bf16, tag="z", bufs=2)
                a4 = acc[:, k, 0:DH].rearrange("p (c h) -> p c h", h=H)
                z4 = z.rearrange("p (c h) -> p c h", h=H)
                for h in range(H):
                    nc.scalar.activation(
                        out=z4[:, :, h], in_=a4[:, :, h],
                        func=AF.Relu, scale=rec[:, h:h+1])
                zk = wp.tile([128, 2, 128], bf16, tag="zk", bufs=2)
                for c in range(2):
                    pt = dlp[:, 128 + c*64:128 + (c+1)*64].bitcast(bf16)
                    nc.tensor.transpose(out=pt, in_=z[:, c*128:(c+1)*128],
                                        identity=ident[:])
                    nc.scalar.activation(out=zk[:, c, :], in_=pt,
                                         func=AF.Copy)
                return zk

            def sweep1(L):
                p = L % 2
                for pr in PAIRS:
                    edge_group(pr, 0, tabA[p], dl_all[p], first=True)

            def sweep2(L, last):
                p = L % 2
                np_ = (L + 1) % 2
                for pr in PAIRS:
                    dlp = edge_group(pr, 1, tabB[p], dl_all[p], first=False)
                    for k in pr:
                        zk = finalize(k, dlp)
                        if not last:
                            w_next = w2_sb if L == 1 else w3_sb
                            ps2 = psB.tile([128, RHSW], f32, tag="tps")
                            for c in range(2):
                                nc.tensor.matmul(
                                    out=ps2[:], lhsT=zk[:, c, :],
                                    rhs=w_next[c][:],
                                    start=(c == 0), stop=(c == 1))
                            stage_write(ps2, k, np_, dl_all[np_])
                            if k == NBA - 1:
                                ag_a(np_)
                        else:
                            ps2 = psB.tile([128, RHSW], f32, tag="tps")
                            for c in range(2):
                                nc.tensor.matmul(
                                    out=ps2[:, 0:DH], lhsT=zk[:, c, :],
                                    rhs=wm1_sb[c][:],
                                    start=(c == 0), stop=(c == 1))
                            m1 = wp.tile([128, DH], bf16, tag="m1", bufs=3)
                            nc.scalar.activation(out=m1[:], in_=ps2[:, 0:DH],
                                                 func=AF.Relu)
                            m1t = wp.tile([128, 2, 128], bf16, tag="m1t",
                                          bufs=3)
                            for c in range(2):
                                pt = dlp[:, 320 + c*64:320 + (c+1)*64].bitcast(
                                    bf16)
                                nc.tensor.transpose(
                                    out=pt, in_=m1[:, c*128:(c+1)*128],
                                    identity=ident[:])
                                nc.scalar.activation(out=m1t[:, c, :],
                                                     in_=pt, func=AF.Copy)
                            po = psB.tile([128, RHSW], f32, tag="tps")
                            for c in range(2):
                                nc.tensor.matmul(
                                    out=po[:, 0:OUTD], lhsT=m1t[:, c, :],
                                    rhs=wm2_sb[c][:],
                                    start=(c == 0), stop=(c == 1))
                            ob = wp.tile([128, OUTD], f32, tag="ob", bufs=3)
                            nc.scalar.activation(out=ob[:], in_=po[:, 0:OUTD],
                                                 func=AF.Copy)
                            nc.sync.dma_start(out=out[k*128:(k+1)*128, :],
                                              in_=ob[:])
                if not last:
                    ag_b(np_)

            # layer-1 transform, computed redundantly for ALL cores' rows
            # straight from the replicated x: table A rows first (so sweep1
            # can start), then per-shard dl, then table B rows. One input
            # DMA per core chunk and one output DMA per (core, half).
            def l1_half(cr, blocks, tab_t, row0):
                xcore = wp.tile([128, SHP], bf16, tag="xcore", bufs=1)
                nc.scalar.dma_start(out=xcore[:],
                                    in_=xT8[:, cr*SHP:(cr+1)*SHP])
                nblk = len(blocks)
                hbX = wp.tile([128, NBA, PAYU], pdt, tag="hbX", bufs=2)
                for i, k in enumerate(blocks):
                    ps2 = psB.tile([128, RHSW], f32, tag="tps")
                    nc.tensor.matmul(out=ps2[:],
                                     lhsT=xcore[:, k*128:(k+1)*128],
                                     rhs=w1_sb[:], start=True, stop=True)
                    nc.scalar.activation(out=hbX[:, i, 0:256],
                                         in_=ps2[:, 0:256], func=AF.Copy)
                    nc.vector.tensor_copy(
                        out=hbX[:, i, SLO:SLO+SLW].bitcast(f32),
                        in_=ps2[:, 256:260])
                nc.sync.dma_start(
                    out=tab_t[row0:row0 + nblk*128, 0:PAYU].rearrange(
                        "(b p) c -> p b c", p=128),
                    in_=hbX[:, 0:nblk, :])

            for cr in range(NC8):
                l1_half(cr, range(NBA), tabA[1], cr * SPA)
            for k in range(NB):   # own-shard dl (layer 1)
                psd = psB.tile([128, RHSW], f32, tag="tps")
                nc.tensor.matmul(out=psd[:, 0:4],
                                 lhsT=zt_x[:, k*128:(k+1)*128],
                                 rhs=w1_sb[:, 260:264], start=True, stop=True)
                nc.vector.tensor_copy(out=dl_all[1][:, k, :],
                                      in_=psd[:, 0:4])
            for cr in range(NC8):
                l1_half(cr, range(NBA, NB), tabB[1], cr * SPB)

            sweep1(1)
            sweep2(1, last=False)   # fuses transform 2 -> set 0, AG(2)
            sweep1(0)
            sweep2(0, last=False)   # fuses transform 3 -> set 1, AG(3)
            sweep1(1)
            sweep2(1, last=True)    # fuses MLP head -> out
    nc.finalize()
    return nc


def kernel(x, edge_index, W1, as1, ad1, b1, W2, as2, ad2, b2, W3, as3, ad3, b3,
           Wm1, bm1, Wm2, bm2):
    global LAST_EXEC_NS
    _install_ntff_hook()

    bfdt = mybir.dt.np(mybir.dt.bfloat16)
    x = np.asarray(x, dtype=np.float32)
    s_max, base_qk, SBT, idxP, dlocT, ohT = _prep_edges(edge_index)

    p1s, p1d = _pack_attn(np.asarray(as1, np.float32), np.asarray(ad1, np.float32))
    p2s, p2d = _pack_attn(np.asarray(as2, np.float32), np.asarray(ad2, np.float32))
    p3s, p3d = _pack_attn(np.asarray(as3, np.float32), np.asarray(ad3, np.float32))
    W1 = np.asarray(W1, np.float32); W2 = np.asarray(W2, np.float32)
    W3 = np.asarray(W3, np.float32)

    # head-minor (c,h) permutation: new feature f=c*4+h <- old h*64+c
    perm = np.empty(DH, dtype=np.int64)
    for h in range(H):
        for c in range(HID):
            perm[c * H + h] = h * HID + c

    W2r = W2[perm, :]
    W3r = W3[perm, :]
    W1e = np.concatenate([W1[:, perm], W1 @ p1s, W1 @ p1d], axis=1).astype(bfdt)
    W2e = np.concatenate([W2r[:, perm], W2r @ p2s, W2r @ p2d], axis=1).astype(bfdt)
    W3e = np.concatenate([W3r[:, perm], W3r @ p3s, W3r @ p3d], axis=1).astype(bfdt)

    iotaF = np.tile(np.arange(128, dtype=np.float32)[None, :], (128, 1))
    Wm1b = np.asarray(Wm1, np.float32)[perm, :].astype(bfdt)
    Wm2b = np.asarray(Wm2, np.float32).astype(bfdt)

    # full per-core-padded transposed x, identical on every core
    xs8 = np.zeros((NC8 * SHP, F0), dtype=np.float32)
    for c in range(NC8):
        xs8[c*SHP:c*SHP + SH] = x[c*SH:(c+1)*SH]
    xT8 = np.ascontiguousarray(xs8.T).astype(bfdt)

    in_maps = []
    for c in range(NC8):
        xs = np.zeros((SHP, F0), dtype=np.float32)
        xs[:SH] = x[c*SH:(c+1)*SH]
        in_maps.append({
            "xT": np.ascontiguousarray(xs.T).astype(bfdt),
            "xT8": xT8,
            "W1e": W1e, "W2e": W2e, "W3e": W3e,
            "Wm1": Wm1b, "Wm2": Wm2b,
            "idxP": idxP[c], "dlocT": dlocT[c], "ohT": ohT[c],
            "iotaF": iotaF,
        })

    nc = _build(s_max, base_qk, SBT)
    trace = os.environ.get("KERNEL_TRACE", "0") == "1"
    res = run_bass_kernel_spmd(nc, in_maps, list(range(NC8)), trace=trace)
    LAST_EXEC_NS = res.exec_time_ns

    out = np.concatenate([res.results[c]["out"][:SH] for c in range(NC8)], axis=0)
    return out.astype(np.float32)


# revision 27
# speedup vs baseline: 1.0263x; 1.0263x over previous
"""3-layer GAT + MLP head on trn2, node-sharded across 8 NeuronCores.

Strategy: dst nodes partitioned 8 ways (6250/core, padded to 6272). Layer 1's
transform is computed REDUNDANTLY on every core straight from the replicated
input x (writes the full payload tables locally - no collective, no startup
stall). Layers 2/3 replicate their payload tables with TWO AllGathers per
layer (source rows split 25/24 blocks per core into tables A and B), issued
mid-sweep so they always overlap edge compute.

The edge phase is q-major two-sweep: sweep 1 handles all table-A edge groups,
accumulating one-hot scatter matmuls into an SBUF f32 accumulator; sweep 2
handles table-B groups, finalizes each dst block (softmax denominators ride
the scatter matmul; normalization is a per-partition scaled relu on the
scalar engine), and fuses the NEXT layer's transform + staging + AllGather
triggers. Next layer's sweep 1 needs only table A (long landed) and its work
covers AllGather-B still in flight.

Edge groups span PAIRS of dst blocks (super-groups) to halve per-group
instruction overhead; gather calls span the pair, capped at 8 sub-blocks
(1024 descriptors, the SWDGE ring limit). Within each (core,q,block) group
edges are sorted by source row so the hardware dma_gather walks HBM in
ascending order. Per-edge dst logits come from a transposed one-hot matmul
(host-shipped ohT) whose outputs ride spare columns of the same PSUM banks
as the scatter accumulation.

Features use a head-minor (c,h) layout (f = c*4 + h, via host-side weight
permutations) so the per-edge em*h multiply has a contiguous innermost run
of 4 bf16 values and qualifies for the DVE 2x performance mode.
"""
import sys, os, types
sys.path.insert(0, '/opt/trn_rl_repo')
import numpy as np
import concourse.bass as bass
import concourse.bacc as bacc
import concourse.tile as tile
from concourse import mybir
from concourse import bass_utils
from concourse.bass_utils import run_bass_kernel_spmd
from concourse.masks import make_identity

N = 50000
F0 = 128
HID = 64
H = 4
DH = 256          # H*HID
OUTD = 40
NEG = 0.2
NC8 = 8
SH = 6250         # dst nodes per core
NB = 49           # 128-node blocks per core
SHP = NB * 128    # 6272
NBA = 25          # blocks in stage half A
NBB = 24          # blocks in stage half B
SPA = NBA * 128   # 3200 rows per core in half A
SPB = NBB * 128   # 3072
ROWSA = NC8 * SPA  # 25600 rows in table A (int16-indexable)
ROWSB = NC8 * SPB  # 24576
FP8 = False       # fp8 payload rows (512B) vs bf16 (768B)
PAYW = 512 if FP8 else 384   # payload row width in elements (%256B rule)
PAYU = 280 if FP8 else 264   # staged columns: h 256 | em slot | sl bits
SLO = 264 if FP8 else 256    # payload col offset of sl (f32 bits)
SLW = 16 if FP8 else 8       # sl width in payload elements
SCATW = 260       # scatter-matmul rhs window: h*em(256) | em(4)
RHSW = 264        # transform psum window: h(256) | sl(4) | dl(4)
MAXSB = 8         # sub-blocks per gather call (1024 descs = hard ring cap)
DLC = 320         # f32 col offset of dl results inside each scatter PSUM bank

LAST_EXEC_NS = None


def _install_ntff_hook():
    if "antenv.axon_hooks" in sys.modules:
        return
    try:
        import antenv
        from trn_agent_boot.trn_boot import _ntff_profile_via_ctypes
        hook = _ntff_profile_via_ctypes('/opt/axon/libaxon_pjrt.so')
    except Exception:
        hook = None
    m = types.ModuleType("antenv.axon_hooks")
    m.get_axon_ntff_profile_hook = lambda: hook
    m.set_axon_ntff_profile_hook = lambda h: None
    sys.modules["antenv.axon_hooks"] = m
    bass_utils.upload_artifacts = lambda d: f"local:{d}"


PAIRS = [(k, k + 1) for k in range(0, NB - 1, 2)] + [(NB - 1,)]


def _prep_edges(edge_index):
    src = np.asarray(edge_index[0], dtype=np.int64)
    dst = np.asarray(edge_index[1], dtype=np.int64)
    loop = np.arange(N, dtype=np.int64)
    src = np.concatenate([src, loop])
    dst = np.concatenate([dst, loop])

    core = dst // SH
    ldst = dst - core * SH
    blk = ldst // 128
    dloc = (ldst - blk * 128).astype(np.float32)
    scr = src // SH
    r = src - scr * SH
    q = (r >= SPA).astype(np.int64)
    rel = np.where(q == 0, scr * SPA + r, scr * SPB + (r - SPA)).astype(np.int16)

    # uniform (over cores) sub-block counts per (q, k)
    key = core * (2 * NB) + q * NB + blk
    counts = np.bincount(key, minlength=NC8 * 2 * NB).reshape(NC8, 2, NB)
    s_max = np.ceil(counts.max(axis=0) / 128).astype(np.int64)  # [2, NB]
    s_max = np.maximum(s_max, 1)
    # one extra all-pad sub-block on block 48 (q=0): carries fake self-edges
    # for the 22 pad dst slots so their softmax denominators stay finite (a
    # NaN there would poison the dl matmul contraction for the whole block)
    s_max[0][NB - 1] += 1

    # slot layout: pair-major, then q, then block-within-pair, so the two
    # blocks of a super-group have contiguous sub-block slots per q
    base_qk = np.zeros((2, NB), dtype=np.int64)
    acc = 0
    for pr in PAIRS:
        for qq in range(2):
            for k in pr:
                base_qk[qq][k] = acc
                acc += int(s_max[qq][k])
    SBT = int(acc)
    base_flat = np.zeros(2 * NB, dtype=np.int64)   # indexed by q*NB + k
    for qq in range(2):
        for k in range(NB):
            base_flat[qq * NB + k] = base_qk[qq][k]

    # sort within each (core,q,block) group by source row for HBM locality
    order = np.lexsort((rel, key))
    key_s = key[order]
    gcounts = np.bincount(key_s, minlength=NC8 * 2 * NB)
    gstart = np.zeros(NC8 * 2 * NB + 1, dtype=np.int64)
    gstart[1:] = np.cumsum(gcounts)
    pos = np.arange(len(key_s)) - gstart[key_s]
    qk = key_s % (2 * NB)
    slot = base_flat[qk] * 128 + pos
    core_s = key_s // (2 * NB)

    # empty slots gather row 0 (the ucode mishandles mid-call negative
    # indices, so -1 skipping is not usable); the dloc>=0 mask on alpha
    # still forces em=1 there so slot contents never compound
    pay_idx = np.zeros((NC8, SBT * 128), dtype=np.int16)
    dloc_a = np.full((NC8, SBT * 128), -1.0, dtype=np.float32)
    pay_idx[core_s, slot] = rel[order]
    dloc_a[core_s, slot] = dloc[order]
    # fake self-edges for block-48 pad dst slots (see s_max bump above)
    pad0 = (base_qk[0][NB - 1] + s_max[0][NB - 1] - 1) * 128
    npad = SHP - SH  # 22
    dloc_a[:, pad0:pad0 + npad] = np.arange(128 - npad, 128, dtype=np.float32)
    pay_idx[:, pad0:pad0 + npad] = 0

    # wrapped int16 idx layout: idx i of a call at [i%16, i//16], replicated
    # across the 8 gpsimd-core stripes of 16 partitions each
    idxP = np.ascontiguousarray(np.tile(
        pay_idx.reshape(NC8, SBT * 8, 16).transpose(0, 2, 1), (1, 8, 1)))
    dlocT = np.ascontiguousarray(
        dloc_a.reshape(NC8, SBT, 128).transpose(0, 2, 1))  # [NC8, 128, SBT]
    # host-built transposed one-hot (partition = dst slot, free = edge slot),
    # bf16 0/1: the lhsT operand of the per-edge dl matmul
    bfdt = mybir.dt.np(mybir.dt.bfloat16)
    ohT = (dloc_a.reshape(NC8, SBT, 128)[:, None, :, :]
           == np.arange(128, dtype=np.float32)[None, :, None, None])
    ohT = np.ascontiguousarray(ohT).astype(bfdt).reshape(NC8, 128, SBT * 128)
    return s_max, base_qk, SBT, idxP, dlocT, ohT


def _pack_attn(a_s, a_d):
    p_s = np.zeros((DH, H), dtype=np.float32)
    p_d = np.zeros((DH, H), dtype=np.float32)
    for h in range(H):
        p_s[h * HID:(h + 1) * HID, h] = a_s[h]
        p_d[h * HID:(h + 1) * HID, h] = a_d[h]
    return p_s, p_d


def _build(s_max, base_qk, SBT):
    f32 = mybir.dt.float32
    bf16 = mybir.dt.bfloat16
    pdt = mybir.dt.float8e4 if FP8 else mybir.dt.bfloat16
    i16 = mybir.dt.int16
    AF = mybir.ActivationFunctionType
    nc = bacc.Bacc("TRN2", target_bir_lowering=False, debug=False,
                   num_swdge_queues=4)

    # max sub-blocks of a (q, pair) super-group
    S2MX = max(sum(int(s_max[qq][k]) for k in pr)
               for pr in PAIRS for qq in range(2))

    xT = nc.dram_tensor("xT", [F0, SHP], bf16, kind="ExternalInput")
    xT8 = nc.dram_tensor("xT8", [F0, NC8 * SHP], bf16, kind="ExternalInput")
    W1e = nc.dram_tensor("W1e", [F0, RHSW], bf16, kind="ExternalInput")
    W2e = nc.dram_tensor("W2e", [DH, RHSW], bf16, kind="ExternalInput")
    W3e = nc.dram_tensor("W3e", [DH, RHSW], bf16, kind="ExternalInput")
    Wm1 = nc.dram_tensor("Wm1", [DH, DH], bf16, kind="ExternalInput")
    Wm2 = nc.dram_tensor("Wm2", [DH, OUTD], bf16, kind="ExternalInput")
    idxP_d = nc.dram_tensor("idxP", [128, SBT * 8], i16, kind="ExternalInput")
    dlocT = nc.dram_tensor("dlocT", [128, SBT], f32, kind="ExternalInput")
    ohT_d = nc.dram_tensor("ohT", [128, SBT * 128], bf16,
                           kind="ExternalInput")
    iotaF = nc.dram_tensor("iotaF", [128, 128], f32, kind="ExternalInput")
    out = nc.dram_tensor("out", [SHP, OUTD], f32, kind="ExternalOutput")

    stageA = [nc.dram_tensor(f"stageA{p}", [SPA, PAYW], pdt) for p in range(2)]
    stageB = [nc.dram_tensor(f"stageB{p}", [SPB, PAYW], pdt) for p in range(2)]
    tabA = [nc.dram_tensor(f"tabA{p}", [ROWSA, PAYW], pdt,
                           addr_space="Shared") for p in range(2)]
    tabB = [nc.dram_tensor(f"tabB{p}", [ROWSB, PAYW], pdt,
                           addr_space="Shared") for p in range(2)]

    with tile.TileContext(nc) as tc:
        with tc.tile_pool(name="const", bufs=1) as cp, \
             tc.tile_pool(name="work", bufs=2) as wp, \
             tc.tile_pool(name="zt", bufs=1) as zp, \
             tc.tile_pool(name="psA", bufs=2, space="PSUM") as psA, \
             tc.tile_pool(name="psB", bufs=2, space="PSUM") as psB, \
             tc.tile_pool(name="psC", bufs=2, space="PSUM") as psC:

            from concourse import library_config
            ident = cp.tile([128, 128], bf16)
            make_identity(nc, ident[:])
            nc.gpsimd.load_library(library_config.mlp)
            iota_sb = cp.tile([128, 128], f32)
            nc.sync.dma_start(out=iota_sb[:], in_=iotaF[:])
            cNEG = cp.tile([128, 4], f32)
            nc.gpsimd.memset(cNEG[:], NEG)
            dloc_sb = cp.tile([128, SBT], f32)
            nc.sync.dma_start(out=dloc_sb[:], in_=dlocT[:])
            ixp_sb = cp.tile([128, SBT * 8], i16)
            nc.sync.dma_start(out=ixp_sb[:], in_=idxP_d[:])

            w1_sb = cp.tile([128, RHSW], bf16)
            nc.sync.dma_start(out=w1_sb[:], in_=W1e[:])
            w2_sb = [cp.tile([128, RHSW], bf16, tag=f"w2_{c}", name=f"w2_{c}")
                     for c in range(2)]
            w3_sb = [cp.tile([128, RHSW], bf16, tag=f"w3_{c}", name=f"w3_{c}")
                     for c in range(2)]
            wm1_sb = [cp.tile([128, DH], bf16, tag=f"wm1_{c}", name=f"wm1_{c}")
                      for c in range(2)]
            wm2_sb = [cp.tile([128, OUTD], bf16, tag=f"wm2_{c}", name=f"wm2_{c}")
                      for c in range(2)]
            for c in range(2):
                nc.sync.dma_start(out=w2_sb[c][:], in_=W2e[c*128:(c+1)*128, :])
                nc.sync.dma_start(out=w3_sb[c][:], in_=W3e[c*128:(c+1)*128, :])
                nc.sync.dma_start(out=wm1_sb[c][:], in_=Wm1[c*128:(c+1)*128, :])
                nc.sync.dma_start(out=wm2_sb[c][:], in_=Wm2[c*128:(c+1)*128, :])

            zt_x = zp.tile([128, SHP], bf16, tag="ztx", name="ztx")
            nc.sync.dma_start(out=zt_x[:], in_=xT[:])
            dl_all = [zp.tile([128, NB, 4], bf16, tag=f"dl{p}", name=f"dl{p}")
                      for p in range(2)]
            acc = zp.tile([128, NB, SCATW], f32, tag="acc", name="acc")

            qrr = [0]

            def stage_write(ps2, k, p, dl_tile):
                """Copy a transform PSUM block into payload staging + dl."""
                hb2 = wp.tile([128, PAYU], pdt, tag="hb2", bufs=3)
                nc.scalar.activation(out=hb2[:, 0:256], in_=ps2[:, 0:256],
                                     func=AF.Copy)
                nc.vector.tensor_copy(
                    out=hb2[:, SLO:SLO+SLW].bitcast(f32),
                    in_=ps2[:, 256:260])
                nc.vector.tensor_copy(out=dl_tile[:, k, :],
                                      in_=ps2[:, 260:264])
                if k < NBA:
                    nc.sync.dma_start(
                        out=stageA[p][k*128:(k+1)*128, 0:PAYU], in_=hb2[:])
                else:
                    kk = k - NBA
                    nc.sync.dma_start(
                        out=stageB[p][kk*128:(kk+1)*128, 0:PAYU], in_=hb2[:])

            def ag_a(p):
                nc.gpsimd.collective_compute(
                    "AllGather", mybir.AluOpType.bypass,
                    replica_groups=[list(range(NC8))],
                    ins=[stageA[p][:]], outs=[tabA[p][:]],
                )

            def ag_b(p):
                nc.gpsimd.collective_compute(
                    "AllGather", mybir.AluOpType.bypass,
                    replica_groups=[list(range(NC8))],
                    ins=[stageB[p][:]], outs=[tabB[p][:]],
                )

            def edge_group(pr, qq, tab, dl_tile, first):
                """Process the (q, block-pair) super-group: gather payload
                rows, per-edge softmax weights, scatter into acc."""
                ss = [int(s_max[qq][k]) for k in pr]
                s2 = sum(ss)
                b0 = int(base_qk[qq][pr[0]])
                ohTt = wp.tile([128, S2MX * 128], bf16, tag="ohT", bufs=2)
                nc.scalar.dma_start(
                    out=ohTt[:, 0:s2*128], in_=ohT_d[:, b0*128:(b0+s2)*128])
                pay = wp.tile([128, S2MX, PAYW], pdt, tag="pay", bufs=3)
                ncalls = -(-s2 // MAXSB)
                s0 = 0
                for c in range(ncalls):
                    nblk = s2 // ncalls + (1 if c < s2 % ncalls else 0)
                    nc.gpsimd.dma_gather(
                        pay[:, s0:s0+nblk, :], tab[:],
                        ixp_sb[:, (b0+s0)*8:(b0+s0+nblk)*8], nblk * 128,
                        nblk * 128, PAYW, queue_num=qrr[0] % 4)
                    qrr[0] += 1
                    s0 += nblk
                # per-edge dl via transposed one-hot matmul (own psum bank;
                # its spare space also hosts the finalize transposes)
                ps = psA.tile([128, 2, 512], f32, tag="eacc")
                dlp = psC.tile([128, 512], f32, tag="po")
                off = 0
                for i, k in enumerate(pr):
                    for j in range(ss[i]):
                        nc.tensor.matmul(
                            out=dlp[:, (off+j)*4:(off+j+1)*4],
                            lhsT=ohTt[:, (off+j)*128:(off+j+1)*128],
                            rhs=dl_tile[:, k, :], start=True, stop=True)
                    off += ss[i]
                # em = exp(leakyrelu(sl + dl)); lrelu on DVE
                alw = wp.tile([128, S2MX, 4], f32, tag="alw", bufs=2)
                al2 = wp.tile([128, S2MX, 4], f32, tag="al2", bufs=2)
                off = 0
                for i, k in enumerate(pr):
                    nc.vector.tensor_tensor(
                        out=alw[:, off:off+ss[i], :],
                        in0=pay[:, off:off+ss[i], SLO:SLO+SLW].bitcast(f32),
                        in1=dlp[:, off*4:(off+ss[i])*4].rearrange(
                            "p (j c) -> p j c", j=ss[i]),
                        op=mybir.AluOpType.add)
                    off += ss[i]
                nc.vector.tensor_tensor(
                    out=al2[:, 0:s2, :], in0=alw[:, 0:s2, :],
                    in1=cNEG[:, 0:1, None].to_broadcast([128, s2, 4]),
                    op=mybir.AluOpType.mult)
                nc.vector.tensor_tensor(
                    out=alw[:, 0:s2, :], in0=alw[:, 0:s2, :],
                    in1=al2[:, 0:s2, :], op=mybir.AluOpType.max)
                nc.scalar.activation(
                    out=pay[:, 0:s2, 256:260], in_=alw[:, 0:s2, :],
                    func=AF.Exp)
                # head-minor (c,h) layout: em broadcast has a contiguous
                # 4-wide innermost run -> DVE 2x mode
                pay4 = pay[:, 0:s2, 0:DH].rearrange(
                    "p j (c h) -> p j c h", h=H)
                nc.vector.tensor_tensor(
                    out=pay4, in0=pay4,
                    in1=pay[:, 0:s2, None, 256:260].to_broadcast(
                        [128, s2, HID, H]),
                    op=mybir.AluOpType.mult)
                ohw = wp.tile([128, S2MX, 128], bf16, tag="ohw", bufs=2)
                nc.vector.tensor_tensor(
                    out=ohw[:, 0:s2, :],
                    in0=dloc_sb[:, b0:b0+s2, None].to_broadcast([128, s2, 128]),
                    in1=iota_sb[:, None, :].to_broadcast([128, s2, 128]),
                    op=mybir.AluOpType.is_equal)
                off = 0
                for i, k in enumerate(pr):
                    for j in range(ss[i]):
                        nc.tensor.matmul(
                            out=ps[:, i, 0:SCATW], lhsT=ohw[:, off+j, :],
                            rhs=pay[:, off+j, 0:SCATW],
                            start=(j == 0), stop=(j == ss[i] - 1))
                    off += ss[i]
                    if first:
                        nc.scalar.activation(out=acc[:, k, :],
                                             in_=ps[:, i, 0:SCATW],
                                             func=AF.Copy)
                    else:
                        nc.vector.tensor_tensor(
                            out=acc[:, k, :], in0=acc[:, k, :],
                            in1=ps[:, i, 0:SCATW], op=mybir.AluOpType.add)
                return dlp

            def finalize(k, dlp):
                """z = relu(acc_h * (1/denom_h)); returns zk transposed chunks."""
                rec = wp.tile([128, 4], f32, tag="rec", bufs=3)
                nc.vector.reciprocal(out=rec[:], in_=acc[:, k, 256:260])
                z = wp.tile([128, DH], bf16, tag="z", bufs=2)
                a4 = acc[:, k, 0:DH].rearrange("p (c h) -> p c h", h=H)
                z4 = z.rearrange("p (c h) -> p c h", h=H)
                for h in range(H):
                    nc.scalar.activation(
                        out=z4[:, :, h], in_=a4[:, :, h],
                        func=AF.Relu, scale=rec[:, h:h+1])
                zk = wp.tile([128, 2, 128], bf16, tag="zk", bufs=2)
                for c in range(2):
                    pt = dlp[:, 128 + c*64:128 + (c+1)*64].bitcast(bf16)
                    nc.tensor.transpose(out=pt, in_=z[:, c*128:(c+1)*128],
                                        identity=ident[:])
                    nc.scalar.activation(out=zk[:, c, :], in_=pt,
                                         func=AF.Copy)
                return zk

            def sweep1(L):
                p = L % 2
                for pr in PAIRS:
                    edge_group(pr, 0, tabA[p], dl_all[p], first=True)

            def sweep2(L, last):
                p = L % 2
                np_ = (L + 1) % 2
                for pr in PAIRS:
                    dlp = edge_group(pr, 1, tabB[p], dl_all[p], first=False)
                    for k in pr:
                        zk = finalize(k, dlp)
                        if not last:
                            w_next = w2_sb if L == 1 else w3_sb
                            ps2 = psB.tile([128, RHSW], f32, tag="tps")
                            for c in range(2):
                                nc.tensor.matmul(
                                    out=ps2[:], lhsT=zk[:, c, :],
                                    rhs=w_next[c][:],
                                    start=(c == 0), stop=(c == 1))
                            stage_write(ps2, k, np_, dl_all[np_])
                            if k == NBA - 1:
                                ag_a(np_)
                        else:
                            ps2 = psB.tile([128, RHSW], f32, tag="tps")
                            for c in range(2):
                                nc.tensor.matmul(
                                    out=ps2[:, 0:DH], lhsT=zk[:, c, :],
                                    rhs=wm1_sb[c][:],
                                    start=(c == 0), stop=(c == 1))
                            m1 = wp.tile([128, DH], bf16, tag="m1", bufs=3)
                            nc.scalar.activation(out=m1[:], in_=ps2[:, 0:DH],
                                                 func=AF.Relu)
                            m1t = wp.tile([128, 2, 128], bf16, tag="m1t",
                                          bufs=3)
                            for c in range(2):
                                pt = dlp[:, 320 + c*64:320 + (c+1)*64].bitcast(
                                    bf16)
                                nc.tensor.transpose(
                                    out=pt, in_=m1[:, c*128:(c+1)*128],
                                    identity=ident[:])
                                nc.scalar.activation(out=m1t[:, c, :],
                                                     in_=pt, func=AF.Copy)
                            po = psB.tile([128, RHSW], f32, tag="tps")
                            for c in range(2):
                                nc.tensor.matmul(
                                    out=po[:, 0:OUTD], lhsT=m1t[:, c, :],
                                    rhs=wm2_sb[c][:],
                                    start=(c == 0), stop=(c == 1))
                            ob = wp.tile([128, OUTD], f32, tag="ob", bufs=3)
                            nc.scalar.activation(out=ob[:], in_=po[:, 0:OUTD],
                                                 func=AF.Copy)
                            nc.sync.dma_start(out=out[k*128:(k+1)*128, :],
                                              in_=ob[:])
                if not last:
                    ag_b(np_)

            # layer-1 transform, computed redundantly for ALL cores' rows
            # straight from the replicated x: table A rows first (so sweep1
            # can start), then per-shard dl, then table B rows. One input
            # DMA per core chunk and one output DMA per (core, half).
            def l1_half(cr, blocks, tab_t, row0):
                xcore = wp.tile([128, SHP], bf16, tag="xcore", bufs=1)
                nc.scalar.dma_start(out=xcore[:],
                                    in_=xT8[:, cr*SHP:(cr+1)*SHP])
                nblk = len(blocks)
                hbX = wp.tile([128, NBA, PAYU], pdt, tag="hbX", bufs=2)
                for i, k in enumerate(blocks):
                    ps2 = psB.tile([128, RHSW], f32, tag="tps")
                    nc.tensor.matmul(out=ps2[:],
                                     lhsT=xcore[:, k*128:(k+1)*128],
                                     rhs=w1_sb[:], start=True, stop=True)
                    nc.scalar.activation(out=hbX[:, i, 0:256],
                                         in_=ps2[:, 0:256], func=AF.Copy)
                    nc.vector.tensor_copy(
                        out=hbX[:, i, SLO:SLO+SLW].bitcast(f32),
                        in_=ps2[:, 256:260])
                nc.sync.dma_start(
                    out=tab_t[row0:row0 + nblk*128, 0:PAYU].rearrange(
                        "(b p) c -> p b c", p=128),
                    in_=hbX[:, 0:nblk, :])

            for cr in range(NC8):
                l1_half(cr, range(NBA), tabA[1], cr * SPA)
            for k in range(NB):   # own-shard dl (layer 1)
                psd = psB.tile([128, RHSW], f32, tag="tps")
                nc.tensor.matmul(out=psd[:, 0:4],
                                 lhsT=zt_x[:, k*128:(k+1)*128],
                                 rhs=w1_sb[:, 260:264], start=True, stop=True)
                nc.vector.tensor_copy(out=dl_all[1][:, k, :],
                                      in_=psd[:, 0:4])
            for cr in range(NC8):
                l1_half(cr, range(NBA, NB), tabB[1], cr * SPB)

            sweep1(1)
            sweep2(1, last=False)   # fuses transform 2 -> set 0, AG(2)
            sweep1(0)
            sweep2(0, last=False)   # fuses transform 3 -> set 1, AG(3)
            sweep1(1)
            sweep2(1, last=True)    # fuses MLP head -> out
    nc.finalize()
    return nc


def kernel(x, edge_index, W1, as1, ad1, b1, W2, as2, ad2, b2, W3, as3, ad3, b3,
           Wm1, bm1, Wm2, bm2):
    global LAST_EXEC_NS
    _install_ntff_hook()

    bfdt = mybir.dt.np(mybir.dt.bfloat16)
    x = np.asarray(x, dtype=np.float32)
    s_max, base_qk, SBT, idxP, dlocT, ohT = _prep_edges(edge_index)

    p1s, p1d = _pack_attn(np.asarray(as1, np.float32), np.asarray(ad1, np.float32))
    p2s, p2d = _pack_attn(np.asarray(as2, np.float32), np.asarray(ad2, np.float32))
    p3s, p3d = _pack_attn(np.asarray(as3, np.float32), np.asarray(ad3, np.float32))
    W1 = np.asarray(W1, np.float32); W2 = np.asarray(W2, np.float32)
    W3 = np.asarray(W3, np.float32)

    # head-minor (c,h) permutation: new feature f=c*4+h <- old h*64+c
    perm = np.empty(DH, dtype=np.int64)
    for h in range(H):
        for c in range(HID):
            perm[c * H + h] = h * HID + c

    W2r = W2[perm, :]
    W3r = W3[perm, :]
    W1e = np.concatenate([W1[:, perm], W1 @ p1s, W1 @ p1d], axis=1).astype(bfdt)
    W2e = np.concatenate([W2r[:, perm], W2r @ p2s, W2r @ p2d], axis=1).astype(bfdt)
    W3e = np.concatenate([W3r[:, perm], W3r @ p3s, W3r @ p3d], axis=1).astype(bfdt)

    iotaF = np.tile(np.arange(128, dtype=np.float32)[None, :], (128, 1))
    Wm1b = np.asarray(Wm1, np.float32)[perm, :].astype(bfdt)
    Wm2b = np.asarray(Wm2, np.float32).astype(bfdt)

    # full per-core-padded transposed x, identical on every core
    xs8 = np.zeros((NC8 * SHP, F0), dtype=np.float32)
    for c in range(NC8):
        xs8[c*SHP:c*SHP + SH] = x[c*SH:(c+1)*SH]
    xT8 = np.ascontiguousarray(xs8.T).astype(bfdt)

    in_maps = []
    for c in range(NC8):
        xs = np.zeros((SHP, F0), dtype=np.float32)
        xs[:SH] = x[c*SH:(c+1)*SH]
        in_maps.append({
            "xT": np.ascontiguousarray(xs.T).astype(bfdt),
            "xT8": xT8,
            "W1e": W1e, "W2e": W2e, "W3e": W3e,
            "Wm1": Wm1b, "Wm2": Wm2b,
            "idxP": idxP[c], "dlocT": dlocT[c], "ohT": ohT[c],
            "iotaF": iotaF,
        })

    nc = _build(s_max, base_qk, SBT)
    trace = os.environ.get("KERNEL_TRACE", "0") == "1"
    res = run_bass_kernel_spmd(nc, in_maps, list(range(NC8)), trace=trace)
    LAST_EXEC_NS = res.exec_time_ns

    out = np.concatenate([res.results[c]["out"][:SH] for c in range(NC8)], axis=0)
    return out.astype(np.float32)
